# revision 11
# baseline (speedup 1.0000x reference)
"""Trainium2 Bass kernel for EnhancedReconstructionLoss (0.8*MSE + 0.2*SSIM-loss).

Sharding: pure data parallel. Batch 32 -> 8 cores x 4 images (12 planes of
512x512 each). Each core computes partial sums (sum x^2, sum y^2, sum x*y,
sum ssim_map); host combines into the scalar loss.

Per-core pipeline per 512x512 plane:
  - load x,y as 4 aligned [128,512] tiles
  - xx=x^2, yy=y^2 (ScalarE Square, with accum for MSE), xy=x*y (DVE stt with
    accum), zz=xx+yy (GPSIMD)
  - vertical 3-tap box filter via TensorE banded matmul (+1-row edge matmuls
    across tile boundaries) for streams (x, y, zz, xy) -> PSUM
  - copy PSUM->SBUF (walrus: only one PSUM operand per DVE instruction)
  - horizontal 3-tap via two shifted-AP adds (DVE)
  - SSIM pointwise tail via fused scalar_tensor_tensor / activation ops
  - ssim accumulated per chunk; final column reduce -> [128,4] output
"""

import sys
import numpy as np

for _p in ("/opt/trn_rl_repo", "/root/.axon_site/_ro/trn_rl_repo"):
    if _p not in sys.path:
        sys.path.insert(0, _p)

N_CORES = 8
IMG = 512
PLANES = 12          # 4 images x 3 channels per core
# Tiles are shifted by -1 row so cross-tile matmul edges only ever need the
# FIRST rows of the next tile (base partition 0, a hardware requirement).
TILE_ROWS = [(0, 127), (127, 255), (255, 383), (383, 511), (511, 512)]
TILES_PER_PLANE = 5
CHUNKS_PER_PLANE = 4
C1 = 0.01 ** 2
C2 = 0.03 ** 2
EPS = 1e-8
S9 = 1.0 / 9.0

# --- tuning knobs ---
CFG = {
    "copy_engine": "act",     # engine for PSUM->SBUF copy of V: 'act' | 'dve'
    "tap_dtype": "f32",       # dtype of Vs/A/S tap chain: 'f32' | 'bf16'
    "gp_ops": ("zz", "n2", "den"),  # ops offloaded to gpsimd
    "dma_eng": "sync",
}

_compiled = None


def _build_nc():
    from contextlib import ExitStack
    import concourse.bass as bass
    import concourse.tile as tile
    from concourse import bacc, mybir

    f32 = mybir.dt.float32
    bf16 = mybir.dt.bfloat16
    tap_dt = f32 if CFG["tap_dtype"] == "f32" else bf16
    Alu = mybir.AluOpType
    Act = mybir.ActivationFunctionType

    nc = bacc.Bacc("TRN2", target_bir_lowering=False, debug=False,
                   enable_asserts=True, num_devices=N_CORES)
    x_d = nc.dram_tensor("x", [PLANES, IMG, IMG], f32, kind="ExternalInput").ap()
    y_d = nc.dram_tensor("y", [PLANES, IMG, IMG], f32, kind="ExternalInput").ap()
    band_d = nc.dram_tensor("band", [257, 128], f32, kind="ExternalInput").ap()
    out_d = nc.dram_tensor("out", [128, 4], f32, kind="ExternalOutput").ap()

    def gp(name):
        return nc.gpsimd if name in CFG["gp_ops"] else nc.vector

    dma = getattr(nc, CFG["dma_eng"])

    with tile.TileContext(nc) as tc, ExitStack() as ctx:
        consts = ctx.enter_context(tc.tile_pool(name="consts", bufs=1))
        inp = ctx.enter_context(tc.tile_pool(name="inp", bufs=2))
        pre = ctx.enter_context(tc.tile_pool(name="pre", bufs=2))
        psum = ctx.enter_context(tc.tile_pool(name="psum", bufs=2, space="PSUM"))
        taps = ctx.enter_context(tc.tile_pool(name="taps", bufs=2))
        tail = ctx.enter_context(tc.tile_pool(name="tail", bufs=2))
        accs = ctx.enter_context(tc.tile_pool(name="accs", bufs=1))

        band_a = consts.tile([128, 128], f32, tag="band_a")   # i-j in {0,1,2}
        dma.dma_start(out=band_a, in_=band_d[0:128, :])
        band_b = consts.tile([127, 128], f32, tag="band_b")   # i-j in {-1,0,1}
        dma.dma_start(out=band_b, in_=band_d[128:255, :])
        e2 = consts.tile([2, 128], f32, tag="e2")
        dma.dma_start(out=e2, in_=band_d[255:257, :])
        e1 = consts.tile([1, 128], f32, tag="e1")
        dma.dma_start(out=e1, in_=band_d[256:257, :])

        nacc = PLANES * TILES_PER_PLANE
        xxacc = accs.tile([128, nacc], f32, tag="xxacc")
        yyacc = accs.tile([128, nacc], f32, tag="yyacc")
        xyacc = accs.tile([128, nacc], f32, tag="xyacc")
        ssacc = accs.tile([128, PLANES * CHUNKS_PER_PLANE], f32, tag="ssacc")
        for a in (xxacc, yyacc, xyacc, ssacc):
            nc.vector.memset(a, 0.0)

        for p in range(PLANES):
            # ---- load plane tiles & pre-pool pointwise ----
            xt, yt, zzt, xyt = [], [], [], []
            for t in range(TILES_PER_PLANE):
                g = p * TILES_PER_PLANE + t
                r0, r1 = TILE_ROWS[t]
                nr = r1 - r0
                x_t = inp.tile([nr, IMG], f32, tag=f"x{t}")
                dma.dma_start(out=x_t, in_=x_d[p, r0:r1, :])
                y_t = inp.tile([nr, IMG], f32, tag=f"y{t}")
                dma.dma_start(out=y_t, in_=y_d[p, r0:r1, :])
                xx_t = pre.tile([nr, IMG], f32, tag="xx")
                nc.scalar.activation(xx_t, x_t, Act.Square,
                                     accum_out=xxacc[0:nr, g:g + 1])
                yy_t = pre.tile([nr, IMG], f32, tag="yy")
                nc.scalar.activation(yy_t, y_t, Act.Square,
                                     accum_out=yyacc[0:nr, g:g + 1])
                xy_t = pre.tile([nr, IMG], f32, tag=f"xy{t}")
                nc.vector.scalar_tensor_tensor(
                    out=xy_t, in0=x_t, scalar=1.0, in1=y_t,
                    op0=Alu.mult, op1=Alu.mult, accum_out=xyacc[0:nr, g:g + 1])
                zz_t = pre.tile([nr, IMG], f32, tag=f"zz{t}")
                gp("zz").tensor_add(zz_t, xx_t, yy_t)
                xt.append(x_t); yt.append(y_t); zzt.append(zz_t); xyt.append(xy_t)

            streams = [xt, yt, zzt, xyt]

            # ---- per output chunk: vertical matmul + horizontal taps + tail ----
            # chunk c covers output rows 128c..128c+127; tile c covers input
            # rows 128c-1..128c+126 (tile 0: 0..126), so the only extra
            # contribution comes from the first row(s) of tile c+1.
            for c in range(CHUNKS_PER_PLANE):
                V = psum.tile([128, 4, IMG], f32, tag="V")
                for s, st in enumerate(streams):
                    main_band = band_b if c == 0 else band_a
                    if c < CHUNKS_PER_PLANE - 1:
                        edge = (e2[0:2, :], st[c + 1][0:2, :])
                    else:
                        edge = (e1[0:1, :], st[c + 1][0:1, :])
                    mms = [(main_band, st[c]), edge]
                    for i, (lhsT, rhs) in enumerate(mms):
                        nc.tensor.matmul(V[:, s, :], lhsT, rhs,
                                         start=(i == 0), stop=(i == len(mms) - 1))

                Vs = taps.tile([128, 4, IMG], tap_dt, tag="Vs")
                if CFG["copy_engine"] == "act":
                    nc.scalar.activation(Vs, V, Act.Copy)
                else:
                    nc.vector.tensor_copy(Vs, V)

                A = taps.tile([128, 4, IMG], tap_dt, tag="A")
                nc.vector.memset(A[:, :, 0:1], 0.0)
                nc.vector.tensor_add(A[:, :, 1:IMG], Vs[:, :, 0:IMG - 1],
                                     Vs[:, :, 1:IMG])
                S = taps.tile([128, 4, IMG], tap_dt, tag="S")
                nc.vector.tensor_add(S[:, :, 0:IMG - 1], A[:, :, 0:IMG - 1],
                                     Vs[:, :, 1:IMG])
                nc.scalar.activation(S[:, :, IMG - 1:IMG], A[:, :, IMG - 1:IMG],
                                     Act.Copy)

                Sx, Sy, Szz, Sxy = (S[:, i, :] for i in range(4))
                qx = tail.tile([128, IMG], f32, tag="qx")
                nc.scalar.activation(qx, Sx, Act.Square, scale=S9)
                qy = tail.tile([128, IMG], f32, tag="qy")
                nc.scalar.activation(qy, Sy, Act.Square, scale=S9)
                P2 = tail.tile([128, IMG], f32, tag="P2")
                nc.vector.scalar_tensor_tensor(out=P2, in0=Sx, scalar=2.0 / 81.0,
                                               in1=Sy, op0=Alu.mult, op1=Alu.mult)
                t1 = tail.tile([128, IMG], f32, tag="t1")
                nc.scalar.activation(t1, Sxy, Act.Copy, scale=2.0 / 9.0, bias=C2)
                U = tail.tile([128, IMG], f32, tag="U")
                nc.vector.scalar_tensor_tensor(out=U, in0=qx, scalar=-C2, in1=qy,
                                               op0=Alu.add, op1=Alu.add)
                den1 = tail.tile([128, IMG], f32, tag="den1")
                nc.vector.scalar_tensor_tensor(out=den1, in0=qx, scalar=C1, in1=qy,
                                               op0=Alu.add, op1=Alu.add)
                den2 = tail.tile([128, IMG], f32, tag="den2")
                nc.vector.scalar_tensor_tensor(out=den2, in0=Szz, scalar=S9,
                                               in1=U, op0=Alu.mult,
                                               op1=Alu.subtract)
                n2 = tail.tile([128, IMG], f32, tag="n2")
                gp("n2").tensor_sub(n2, t1, P2)
                num = tail.tile([128, IMG], f32, tag="num")
                nc.vector.scalar_tensor_tensor(out=num, in0=P2, scalar=C1, in1=n2,
                                               op0=Alu.add, op1=Alu.mult)
                den = tail.tile([128, IMG], f32, tag="den")
                gp("den").tensor_mul(den, den1, den2)
                denp = tail.tile([128, IMG], f32, tag="denp")
                nc.scalar.activation(denp, den, Act.Copy, bias=EPS)
                r = tail.tile([128, IMG], f32, tag="r")
                nc.vector.reciprocal_approx_fast(out=r, in_=denp)
                scr = tail.tile([128, IMG], f32, tag="scr")
                nc.vector.scalar_tensor_tensor(
                    out=scr, in0=num, scalar=1.0, in1=r,
                    op0=Alu.mult, op1=Alu.mult,
                    accum_out=ssacc[:, p * 4 + c:p * 4 + c + 1])

        red = accs.tile([128, 4], f32, tag="red")
        nc.vector.reduce_sum(red[:, 0:1], xxacc, axis=mybir.AxisListType.X)
        nc.vector.reduce_sum(red[:, 1:2], yyacc, axis=mybir.AxisListType.X)
        nc.vector.reduce_sum(red[:, 2:3], xyacc, axis=mybir.AxisListType.X)
        nc.vector.reduce_sum(red[:, 3:4], ssacc, axis=mybir.AxisListType.X)
        dma.dma_start(out=out_d, in_=red)

    nc.compile()
    return nc


def _band_host():
    b = np.zeros((257, 128), np.float32)
    for i in range(128):            # BAND_A: i-j in {0,1,2}
        for j in range(128):
            if i - j in (0, 1, 2):
                b[i, j] = 1.0
    for i in range(127):            # BAND_B: i-j in {-1,0,1}
        for j in range(128):
            if i - j in (-1, 0, 1):
                b[128 + i, j] = 1.0
    b[255, 126] = 1.0               # E2 row 0: next-tile row 128c+127
    b[255, 127] = 1.0
    b[256, 127] = 1.0               # E2 row 1 / E1: row 128c+128
    return b


def _get_compiled():
    global _compiled
    if _compiled is None:
        _compiled = _build_nc()
    return _compiled


def _shard_inputs(reconstruction, target):
    band = _band_host()
    in_maps = []
    for i in range(N_CORES):
        xs = np.ascontiguousarray(
            reconstruction[4 * i:4 * (i + 1)].reshape(PLANES, IMG, IMG),
            dtype=np.float32)
        ys = np.ascontiguousarray(
            target[4 * i:4 * (i + 1)].reshape(PLANES, IMG, IMG),
            dtype=np.float32)
        in_maps.append({"x": xs, "y": ys, "band": band})
    return in_maps


def _combine(results):
    sxx = syy = sxy = sss = 0.0
    for i in range(N_CORES):
        red = results[i]["out"].astype(np.float64)
        sxx += red[:, 0].sum()
        syy += red[:, 1].sum()
        sxy += red[:, 2].sum()
        sss += red[:, 3].sum()
    n = float(N_CORES * PLANES * IMG * IMG)
    mse = (sxx + syy - 2.0 * sxy) / n
    ssim_loss = 1.0 - sss / n
    return np.float32(0.8 * mse + 0.2 * ssim_loss)


def run(reconstruction, target, trace=False):
    from concourse.bass_utils import run_bass_kernel_spmd
    nc = _get_compiled()
    in_maps = _shard_inputs(np.asarray(reconstruction), np.asarray(target))
    res = run_bass_kernel_spmd(nc, in_maps, list(range(N_CORES)), trace=trace)
    return _combine(res.results), res


def kernel(reconstruction, target):
    out, _ = run(reconstruction, target, trace=False)
    return out


# revision 17
# speedup vs baseline: 1.3185x; 1.3185x over previous
"""Trainium2 Bass kernel for EnhancedReconstructionLoss (0.8*MSE + 0.2*SSIM-loss).

Sharding: pure data parallel. Batch 32 -> 8 cores x 4 images (12 planes of
512x512 each). Each core computes partial sums (sum x^2, sum y^2, sum x*y,
sum ssim_map); host combines into the scalar loss.

Per-core pipeline per 512x512 plane:
  - load x,y as 4 aligned [128,512] tiles
  - xx=x^2, yy=y^2 (ScalarE Square, with accum for MSE), xy=x*y (DVE stt with
    accum), zz=xx+yy (GPSIMD)
  - vertical 3-tap box filter via TensorE banded matmul (+1-row edge matmuls
    across tile boundaries) for streams (x, y, zz, xy) -> PSUM
  - copy PSUM->SBUF (walrus: only one PSUM operand per DVE instruction)
  - horizontal 3-tap via two shifted-AP adds (DVE)
  - SSIM pointwise tail via fused scalar_tensor_tensor / activation ops
  - ssim accumulated per chunk; final column reduce -> [128,4] output
"""

import sys
import numpy as np

for _p in ("/opt/trn_rl_repo", "/root/.axon_site/_ro/trn_rl_repo"):
    if _p not in sys.path:
        sys.path.insert(0, _p)

N_CORES = 8
IMG = 512
PLANES = 12          # 4 images x 3 channels per core
# Tiles are shifted by -1 row so cross-tile matmul edges only ever need the
# FIRST rows of the next tile (base partition 0, a hardware requirement).
TILE_ROWS = [(0, 127), (127, 255), (255, 383), (383, 511), (511, 512)]
TILES_PER_PLANE = 5
CHUNKS_PER_PLANE = 4
C1 = 0.01 ** 2
C2 = 0.03 ** 2
EPS = 1e-8
S9 = 1.0 / 9.0

# --- tuning knobs ---
CFG = {
    "copy_engine": "act",     # engine for PSUM->SBUF copy of V: 'act' | 'dve'
    "mm_dtype": "bf16",       # matmul input dtype: 'f32' | 'bf16'
    "tap_dtype": "bf16",      # dtype of Vs/A/S tap chain: 'f32' | 'bf16'
    "gp_ops": ("zz", "n2", "den"),  # ops offloaded to gpsimd
    "cast_engine": "dve",     # engine for x/y bf16 casts: 'dve' | 'gp' | 'act'
    "drop_eps": False,        # skip the +EPS activate (error ~1e-7, saves ACT)
    "dma_eng": "sync",
}

_compiled = None


def _build_nc():
    from contextlib import ExitStack
    import concourse.bass as bass
    import concourse.tile as tile
    from concourse import bacc, mybir

    f32 = mybir.dt.float32
    bf16 = mybir.dt.bfloat16
    tap_dt = f32 if CFG["tap_dtype"] == "f32" else bf16
    mm_dt = f32 if CFG["mm_dtype"] == "f32" else bf16
    Alu = mybir.AluOpType
    Act = mybir.ActivationFunctionType

    nc = bacc.Bacc("TRN2", target_bir_lowering=False, debug=False,
                   enable_asserts=True, num_devices=N_CORES)
    x_d = nc.dram_tensor("x", [PLANES, IMG, IMG], f32, kind="ExternalInput").ap()
    y_d = nc.dram_tensor("y", [PLANES, IMG, IMG], f32, kind="ExternalInput").ap()
    band_d = nc.dram_tensor("band", [257, 128], mm_dt, kind="ExternalInput").ap()
    out_d = nc.dram_tensor("out", [128, 4], f32, kind="ExternalOutput").ap()

    def gp(name):
        return nc.gpsimd if name in CFG["gp_ops"] else nc.vector

    dma = getattr(nc, CFG["dma_eng"])

    with tile.TileContext(nc) as tc, ExitStack() as ctx:
        consts = ctx.enter_context(tc.tile_pool(name="consts", bufs=1))
        inp = ctx.enter_context(tc.tile_pool(name="inp", bufs=2))
        pre = ctx.enter_context(tc.tile_pool(name="pre", bufs=2))
        psum = ctx.enter_context(tc.tile_pool(name="psum", bufs=2, space="PSUM"))
        taps = ctx.enter_context(tc.tile_pool(name="taps", bufs=2))
        tail = ctx.enter_context(tc.tile_pool(name="tail", bufs=2))
        accs = ctx.enter_context(tc.tile_pool(name="accs", bufs=1))

        band_a = consts.tile([128, 128], mm_dt, tag="band_a")  # i-j in {0,1,2}
        dma.dma_start(out=band_a, in_=band_d[0:128, :])
        band_b = consts.tile([127, 128], mm_dt, tag="band_b")  # i-j in {-1,0,1}
        dma.dma_start(out=band_b, in_=band_d[128:255, :])
        e2 = consts.tile([2, 128], mm_dt, tag="e2")
        dma.dma_start(out=e2, in_=band_d[255:257, :])
        e1 = consts.tile([1, 128], mm_dt, tag="e1")
        dma.dma_start(out=e1, in_=band_d[256:257, :])

        nacc = PLANES * TILES_PER_PLANE
        xxacc = accs.tile([128, nacc], f32, tag="xxacc")
        yyacc = accs.tile([128, nacc], f32, tag="yyacc")
        xyacc = accs.tile([128, nacc], f32, tag="xyacc")
        ssacc = accs.tile([128, PLANES * CHUNKS_PER_PLANE], f32, tag="ssacc")
        for a in (xxacc, yyacc, xyacc, ssacc):
            nc.vector.memset(a, 0.0)

        for p in range(PLANES):
            # ---- load plane tiles & pre-pool pointwise ----
            xt, yt, zzt, xyt = [], [], [], []
            for t in range(TILES_PER_PLANE):
                g = p * TILES_PER_PLANE + t
                r0, r1 = TILE_ROWS[t]
                nr = r1 - r0
                x_t = inp.tile([nr, IMG], f32, tag=f"x{t}")
                dma.dma_start(out=x_t, in_=x_d[p, r0:r1, :])
                y_t = inp.tile([nr, IMG], f32, tag=f"y{t}")
                dma.dma_start(out=y_t, in_=y_d[p, r0:r1, :])
                xx_t = pre.tile([nr, IMG], mm_dt, tag="xx")
                nc.scalar.activation(xx_t, x_t, Act.Square,
                                     accum_out=xxacc[0:nr, g:g + 1])
                yy_t = pre.tile([nr, IMG], mm_dt, tag="yy")
                nc.scalar.activation(yy_t, y_t, Act.Square,
                                     accum_out=yyacc[0:nr, g:g + 1])
                xy_t = pre.tile([nr, IMG], mm_dt, tag=f"xy{t}")
                nc.vector.scalar_tensor_tensor(
                    out=xy_t, in0=x_t, scalar=1.0, in1=y_t,
                    op0=Alu.mult, op1=Alu.mult, accum_out=xyacc[0:nr, g:g + 1])
                zz_t = pre.tile([nr, IMG], mm_dt, tag=f"zz{t}")
                gp("zz").tensor_add(zz_t, xx_t, yy_t)
                if CFG["mm_dtype"] == "f32":
                    xb_t, yb_t = x_t, y_t
                else:
                    cast = {"dve": nc.vector, "gp": nc.gpsimd}.get(CFG["cast_engine"])
                    xb_t = pre.tile([nr, IMG], mm_dt, tag=f"xb{t}")
                    yb_t = pre.tile([nr, IMG], mm_dt, tag=f"yb{t}")
                    if cast is None:
                        nc.scalar.activation(xb_t, x_t, Act.Copy)
                        nc.scalar.activation(yb_t, y_t, Act.Copy)
                    else:
                        cast.tensor_copy(xb_t, x_t)
                        cast.tensor_copy(yb_t, y_t)
                xt.append(xb_t); yt.append(yb_t); zzt.append(zz_t); xyt.append(xy_t)

            streams = [xt, yt, zzt, xyt]

            # ---- per output chunk: vertical matmul + horizontal taps + tail ----
            # chunk c covers output rows 128c..128c+127; tile c covers input
            # rows 128c-1..128c+126 (tile 0: 0..126), so the only extra
            # contribution comes from the first row(s) of tile c+1.
            for c in range(CHUNKS_PER_PLANE):
                V = psum.tile([128, 4, IMG], f32, tag="V")
                for s, st in enumerate(streams):
                    main_band = band_b if c == 0 else band_a
                    if c < CHUNKS_PER_PLANE - 1:
                        edge = (e2[0:2, :], st[c + 1][0:2, :])
                    else:
                        edge = (e1[0:1, :], st[c + 1][0:1, :])
                    mms = [(main_band, st[c]), edge]
                    for i, (lhsT, rhs) in enumerate(mms):
                        nc.tensor.matmul(V[:, s, :], lhsT, rhs,
                                         start=(i == 0), stop=(i == len(mms) - 1))

                Vs = taps.tile([128, 4, IMG], tap_dt, tag="Vs")
                if CFG["copy_engine"] == "act":
                    nc.scalar.activation(Vs, V, Act.Copy)
                else:
                    nc.vector.tensor_copy(Vs, V)

                A = taps.tile([128, 4, IMG], tap_dt, tag="A")
                nc.vector.memset(A[:, :, 0:1], 0.0)
                nc.vector.tensor_add(A[:, :, 1:IMG], Vs[:, :, 0:IMG - 1],
                                     Vs[:, :, 1:IMG])
                S = taps.tile([128, 4, IMG], tap_dt, tag="S")
                nc.vector.tensor_add(S[:, :, 0:IMG - 1], A[:, :, 0:IMG - 1],
                                     Vs[:, :, 1:IMG])
                nc.scalar.activation(S[:, :, IMG - 1:IMG], A[:, :, IMG - 1:IMG],
                                     Act.Copy)

                Sx, Sy, Szz, Sxy = (S[:, i, :] for i in range(4))
                qx = tail.tile([128, IMG], f32, tag="qx")
                nc.scalar.activation(qx, Sx, Act.Square, scale=S9)
                qy = tail.tile([128, IMG], f32, tag="qy")
                nc.scalar.activation(qy, Sy, Act.Square, scale=S9)
                P2 = tail.tile([128, IMG], f32, tag="P2")
                nc.vector.scalar_tensor_tensor(out=P2, in0=Sx, scalar=2.0 / 81.0,
                                               in1=Sy, op0=Alu.mult, op1=Alu.mult)
                t1 = tail.tile([128, IMG], f32, tag="t1")
                nc.scalar.activation(t1, Sxy, Act.Copy, scale=2.0 / 9.0, bias=C2)
                U = tail.tile([128, IMG], f32, tag="U")
                nc.vector.scalar_tensor_tensor(out=U, in0=qx, scalar=-C2, in1=qy,
                                               op0=Alu.add, op1=Alu.add)
                den1 = tail.tile([128, IMG], f32, tag="den1")
                nc.vector.scalar_tensor_tensor(out=den1, in0=qx, scalar=C1, in1=qy,
                                               op0=Alu.add, op1=Alu.add)
                den2 = tail.tile([128, IMG], f32, tag="den2")
                nc.vector.scalar_tensor_tensor(out=den2, in0=Szz, scalar=S9,
                                               in1=U, op0=Alu.mult,
                                               op1=Alu.subtract)
                n2 = tail.tile([128, IMG], f32, tag="n2")
                gp("n2").tensor_sub(n2, t1, P2)
                num = tail.tile([128, IMG], f32, tag="num")
                nc.vector.scalar_tensor_tensor(out=num, in0=P2, scalar=C1, in1=n2,
                                               op0=Alu.add, op1=Alu.mult)
                den = tail.tile([128, IMG], f32, tag="den")
                gp("den").tensor_mul(den, den1, den2)
                if CFG["drop_eps"]:
                    denp = den
                else:
                    denp = tail.tile([128, IMG], f32, tag="denp")
                    nc.scalar.activation(denp, den, Act.Copy, bias=EPS)
                r = tail.tile([128, IMG], f32, tag="r")
                nc.vector.reciprocal_approx_fast(out=r, in_=denp)
                scr = tail.tile([128, IMG], f32, tag="scr")
                nc.vector.scalar_tensor_tensor(
                    out=scr, in0=num, scalar=1.0, in1=r,
                    op0=Alu.mult, op1=Alu.mult,
                    accum_out=ssacc[:, p * 4 + c:p * 4 + c + 1])

        red = accs.tile([128, 4], f32, tag="red")
        nc.vector.reduce_sum(red[:, 0:1], xxacc, axis=mybir.AxisListType.X)
        nc.vector.reduce_sum(red[:, 1:2], yyacc, axis=mybir.AxisListType.X)
        nc.vector.reduce_sum(red[:, 2:3], xyacc, axis=mybir.AxisListType.X)
        nc.vector.reduce_sum(red[:, 3:4], ssacc, axis=mybir.AxisListType.X)
        dma.dma_start(out=out_d, in_=red)

    nc.compile()
    return nc


def _band_host():
    b = np.zeros((257, 128), np.float32)
    for i in range(128):            # BAND_A: i-j in {0,1,2}
        for j in range(128):
            if i - j in (0, 1, 2):
                b[i, j] = 1.0
    for i in range(127):            # BAND_B: i-j in {-1,0,1}
        for j in range(128):
            if i - j in (-1, 0, 1):
                b[128 + i, j] = 1.0
    b[255, 126] = 1.0               # E2 row 0: next-tile row 128c+127
    b[255, 127] = 1.0
    b[256, 127] = 1.0               # E2 row 1 / E1: row 128c+128
    return b


def _get_compiled():
    global _compiled
    if _compiled is None:
        _compiled = _build_nc()
    return _compiled


def _shard_inputs(reconstruction, target):
    band = _band_host()
    if CFG["mm_dtype"] == "bf16":
        import ml_dtypes
        band = band.astype(ml_dtypes.bfloat16)
    in_maps = []
    for i in range(N_CORES):
        xs = np.ascontiguousarray(
            reconstruction[4 * i:4 * (i + 1)].reshape(PLANES, IMG, IMG),
            dtype=np.float32)
        ys = np.ascontiguousarray(
            target[4 * i:4 * (i + 1)].reshape(PLANES, IMG, IMG),
            dtype=np.float32)
        in_maps.append({"x": xs, "y": ys, "band": band})
    return in_maps


def _combine(results):
    sxx = syy = sxy = sss = 0.0
    for i in range(N_CORES):
        red = results[i]["out"].astype(np.float64)
        sxx += red[:, 0].sum()
        syy += red[:, 1].sum()
        sxy += red[:, 2].sum()
        sss += red[:, 3].sum()
    n = float(N_CORES * PLANES * IMG * IMG)
    mse = (sxx + syy - 2.0 * sxy) / n
    ssim_loss = 1.0 - sss / n
    return np.float32(0.8 * mse + 0.2 * ssim_loss)


def run(reconstruction, target, trace=False):
    from concourse.bass_utils import run_bass_kernel_spmd
    nc = _get_compiled()
    in_maps = _shard_inputs(np.asarray(reconstruction), np.asarray(target))
    res = run_bass_kernel_spmd(nc, in_maps, list(range(N_CORES)), trace=trace)
    return _combine(res.results), res


def kernel(reconstruction, target):
    out, _ = run(reconstruction, target, trace=False)
    return out


# revision 22
# speedup vs baseline: 1.6036x; 1.2162x over previous
"""Trainium2 Bass kernel for EnhancedReconstructionLoss (0.8*MSE + 0.2*SSIM-loss).

Sharding: pure data parallel. Batch 32 -> 8 cores x 4 images (12 planes of
512x512 each). Each core computes partial sums (sum x^2, sum y^2, sum x*y,
sum ssim_map); host combines into the scalar loss.

Per-core pipeline per 512x512 plane (inputs pre-cast to bf16 on host; all
reductions accumulate in fp32 on-chip):
  - load x,y as 5 row-shifted tiles (rows 128t-1..128t+126) so cross-tile
    vertical-filter edges only need base-partition-0 operands
  - xx=x^2, yy=y^2 (with fp32 accum for MSE), xy=x*y (accum), zz=xx+yy
  - vertical 3-tap box filter via TensorE banded matmul -> PSUM (fp32)
  - PSUM->SBUF bf16 copy (hw: only one PSUM operand per DVE instruction),
    horizontal 3-tap via two shifted-AP adds -> S2[stream, chunk, col]
  - SSIM pointwise tail once per plane at FD=2048 (amortizes per-op overhead)
  - final column reduce of accumulators -> [128,4] output
"""

import sys
import numpy as np

for _p in ("/opt/trn_rl_repo", "/root/.axon_site/_ro/trn_rl_repo"):
    if _p not in sys.path:
        sys.path.insert(0, _p)

N_CORES = 8
IMG = 512
PLANES = 12          # 4 images x 3 channels per core
# Tiles are shifted by -1 row so cross-tile matmul edges only ever need the
# FIRST rows of the next tile (base partition 0, a hardware requirement).
TILE_ROWS = [(0, 127), (127, 255), (255, 383), (383, 511), (511, 512)]
TILES_PER_PLANE = 5
CHUNKS_PER_PLANE = 4
C1 = 0.01 ** 2
C2 = 0.03 ** 2
EPS = 1e-8
S9 = 1.0 / 9.0

# --- tuning knobs ---
CFG = {
    "in_dtype": "bf16",      # dtype of x/y shipped to the device
    "sq_engine": "act",      # xx/yy squares: 'act' | 'dve' (no stt on gpsimd)
    "copy_engine": "act",    # PSUM->SBUF copy of V: 'act' | 'dve'
    "drop_eps": True,        # fold away the +EPS activate (error ~1e-7)
    "gp_tap1_chunks": 0,     # how many of 4 chunks run tap1 on gpsimd
    "dma_eng": "sync",
}

_compiled = None


def _build_nc():
    from contextlib import ExitStack
    import concourse.bass as bass
    import concourse.tile as tile
    from concourse import bacc, mybir

    f32 = mybir.dt.float32
    bf16 = mybir.dt.bfloat16
    in_dt = bf16 if CFG["in_dtype"] == "bf16" else f32
    Alu = mybir.AluOpType
    Act = mybir.ActivationFunctionType

    nc = bacc.Bacc("TRN2", target_bir_lowering=False, debug=False,
                   enable_asserts=True, num_devices=N_CORES)
    x_d = nc.dram_tensor("x", [PLANES, IMG, IMG], in_dt, kind="ExternalInput").ap()
    y_d = nc.dram_tensor("y", [PLANES, IMG, IMG], in_dt, kind="ExternalInput").ap()
    band_d = nc.dram_tensor("band", [257, 128], in_dt, kind="ExternalInput").ap()
    out_d = nc.dram_tensor("out", [128, 4], f32, kind="ExternalOutput").ap()

    dma = getattr(nc, CFG["dma_eng"])

    with tile.TileContext(nc) as tc, ExitStack() as ctx:
        consts = ctx.enter_context(tc.tile_pool(name="consts", bufs=1))
        inp = ctx.enter_context(tc.tile_pool(name="inp", bufs=2))
        pre = ctx.enter_context(tc.tile_pool(name="pre", bufs=2))
        psum = ctx.enter_context(tc.tile_pool(name="psum", bufs=2, space="PSUM"))
        taps = ctx.enter_context(tc.tile_pool(name="taps", bufs=2))
        s2p = ctx.enter_context(tc.tile_pool(name="s2p", bufs=2))
        tail = ctx.enter_context(tc.tile_pool(name="tail", bufs=2))
        accs = ctx.enter_context(tc.tile_pool(name="accs", bufs=1))

        band_a = consts.tile([128, 128], in_dt, tag="band_a")  # i-j in {0,1,2}
        dma.dma_start(out=band_a, in_=band_d[0:128, :])
        band_b = consts.tile([127, 128], in_dt, tag="band_b")  # i-j in {-1,0,1}
        dma.dma_start(out=band_b, in_=band_d[128:255, :])
        e2 = consts.tile([2, 128], in_dt, tag="e2")
        dma.dma_start(out=e2, in_=band_d[255:257, :])
        e1 = consts.tile([1, 128], in_dt, tag="e1")
        dma.dma_start(out=e1, in_=band_d[256:257, :])

        nacc = PLANES * TILES_PER_PLANE
        xxacc = accs.tile([128, nacc], f32, tag="xxacc")
        yyacc = accs.tile([128, nacc], f32, tag="yyacc")
        xyacc = accs.tile([128, nacc], f32, tag="xyacc")
        ssacc = accs.tile([128, PLANES], f32, tag="ssacc")
        for a in (xxacc, yyacc, xyacc, ssacc):
            nc.vector.memset(a, 0.0)

        for p in range(PLANES):
            # ---- load plane tiles & pre-pool pointwise ----
            xt, yt, zzt, xyt = [], [], [], []
            for t in range(TILES_PER_PLANE):
                g = p * TILES_PER_PLANE + t
                r0, r1 = TILE_ROWS[t]
                nr = r1 - r0
                x_t = inp.tile([nr, IMG], in_dt, tag=f"x{t}")
                dma.dma_start(out=x_t, in_=x_d[p, r0:r1, :])
                y_t = inp.tile([nr, IMG], in_dt, tag=f"y{t}")
                dma.dma_start(out=y_t, in_=y_d[p, r0:r1, :])
                xx_t = pre.tile([nr, IMG], in_dt, tag="xx")
                yy_t = pre.tile([nr, IMG], in_dt, tag="yy")
                if CFG["sq_engine"] == "act":
                    nc.scalar.activation(xx_t, x_t, Act.Square,
                                         accum_out=xxacc[0:nr, g:g + 1])
                    nc.scalar.activation(yy_t, y_t, Act.Square,
                                         accum_out=yyacc[0:nr, g:g + 1])
                else:
                    sq = nc.vector
                    sq.scalar_tensor_tensor(
                        out=xx_t, in0=x_t, scalar=1.0, in1=x_t,
                        op0=Alu.mult, op1=Alu.mult,
                        accum_out=xxacc[0:nr, g:g + 1])
                    sq.scalar_tensor_tensor(
                        out=yy_t, in0=y_t, scalar=1.0, in1=y_t,
                        op0=Alu.mult, op1=Alu.mult,
                        accum_out=yyacc[0:nr, g:g + 1])
                xy_t = pre.tile([nr, IMG], in_dt, tag=f"xy{t}")
                nc.vector.scalar_tensor_tensor(
                    out=xy_t, in0=x_t, scalar=1.0, in1=y_t,
                    op0=Alu.mult, op1=Alu.mult, accum_out=xyacc[0:nr, g:g + 1])
                zz_t = pre.tile([nr, IMG], in_dt, tag=f"zz{t}")
                nc.gpsimd.tensor_add(zz_t, xx_t, yy_t)
                xt.append(x_t); yt.append(y_t); zzt.append(zz_t); xyt.append(xy_t)

            streams = [xt, yt, zzt, xyt]

            # S2 holds the fully box-filtered sums for the whole plane:
            # [partition, stream, chunk, col]
            S2 = s2p.tile([128, 4, CHUNKS_PER_PLANE, IMG], bf16, tag="S2")

            # ---- per output chunk: vertical matmul + horizontal taps ----
            # chunk c covers output rows 128c..128c+127; tile c covers input
            # rows 128c-1..128c+126 (tile 0: 0..126), so the only extra
            # contribution comes from the first row(s) of tile c+1.
            for c in range(CHUNKS_PER_PLANE):
                V = psum.tile([128, 4, IMG], f32, tag="V")
                for s, st in enumerate(streams):
                    main_band = band_b if c == 0 else band_a
                    if c < CHUNKS_PER_PLANE - 1:
                        edge = (e2[0:2, :], st[c + 1][0:2, :])
                    else:
                        edge = (e1[0:1, :], st[c + 1][0:1, :])
                    mms = [(main_band, st[c]), edge]
                    for i, (lhsT, rhs) in enumerate(mms):
                        nc.tensor.matmul(V[:, s, :], lhsT, rhs,
                                         start=(i == 0), stop=(i == len(mms) - 1))

                Vs = taps.tile([128, 4, IMG], bf16, tag="Vs")
                if CFG["copy_engine"] == "act":
                    nc.scalar.activation(Vs, V, Act.Copy)
                else:
                    nc.vector.tensor_copy(Vs, V)

                A = taps.tile([128, 4, IMG], bf16, tag="A")
                nc.vector.memset(A[:, :, 0:1], 0.0)
                tap1 = nc.gpsimd if c < CFG["gp_tap1_chunks"] else nc.vector
                tap1.tensor_add(A[:, :, 1:IMG], Vs[:, :, 0:IMG - 1],
                                Vs[:, :, 1:IMG])
                nc.vector.tensor_add(S2[:, :, c, 0:IMG - 1], A[:, :, 0:IMG - 1],
                                     Vs[:, :, 1:IMG])
                nc.scalar.activation(S2[:, :, c, IMG - 1:IMG],
                                     A[:, :, IMG - 1:IMG], Act.Copy)

            # ---- SSIM pointwise tail, whole plane at once (FD = 2048) ----
            FD = CHUNKS_PER_PLANE * IMG
            Sx = S2[:, 0, :, :]
            Sy = S2[:, 1, :, :]
            Szz = S2[:, 2, :, :]
            Sxy = S2[:, 3, :, :]
            qx = tail.tile([128, FD], bf16, tag="qx")
            nc.scalar.activation(qx, Sx, Act.Square, scale=S9)
            qy = tail.tile([128, FD], bf16, tag="qy")
            nc.scalar.activation(qy, Sy, Act.Square, scale=S9)
            P2 = tail.tile([128, FD], bf16, tag="P2")
            nc.vector.scalar_tensor_tensor(out=P2, in0=Sx, scalar=2.0 / 81.0,
                                           in1=Sy, op0=Alu.mult, op1=Alu.mult)
            t1 = tail.tile([128, FD], bf16, tag="t1")
            nc.scalar.activation(t1, Sxy, Act.Copy, scale=2.0 / 9.0, bias=C2)
            den1 = tail.tile([128, FD], bf16, tag="den1")
            nc.vector.scalar_tensor_tensor(out=den1, in0=qx, scalar=C1, in1=qy,
                                           op0=Alu.add, op1=Alu.add)
            U = tail.tile([128, FD], bf16, tag="U")
            nc.vector.tensor_scalar_add(U, den1, -(C1 + C2))
            den2 = tail.tile([128, FD], bf16, tag="den2")
            nc.vector.scalar_tensor_tensor(out=den2, in0=Szz, scalar=S9,
                                           in1=U, op0=Alu.mult, op1=Alu.subtract)
            n2 = tail.tile([128, FD], bf16, tag="n2")
            nc.gpsimd.tensor_sub(n2, t1, P2)
            num = tail.tile([128, FD], bf16, tag="num")
            nc.vector.scalar_tensor_tensor(out=num, in0=P2, scalar=C1, in1=n2,
                                           op0=Alu.add, op1=Alu.mult)
            den = tail.tile([128, FD], f32, tag="den")
            nc.gpsimd.tensor_mul(den, den1, den2)
            if CFG["drop_eps"]:
                denp = den
            else:
                denp = tail.tile([128, FD], f32, tag="denp")
                nc.scalar.activation(denp, den, Act.Copy, bias=EPS)
            r = tail.tile([128, FD], f32, tag="r")
            nc.vector.reciprocal_approx_fast(out=r, in_=denp)
            scr = tail.tile([128, FD], bf16, tag="scr")
            nc.vector.scalar_tensor_tensor(
                out=scr, in0=num, scalar=1.0, in1=r,
                op0=Alu.mult, op1=Alu.mult, accum_out=ssacc[:, p:p + 1])

        red = accs.tile([128, 4], f32, tag="red")
        nc.vector.reduce_sum(red[:, 0:1], xxacc, axis=mybir.AxisListType.X)
        nc.vector.reduce_sum(red[:, 1:2], yyacc, axis=mybir.AxisListType.X)
        nc.vector.reduce_sum(red[:, 2:3], xyacc, axis=mybir.AxisListType.X)
        nc.vector.reduce_sum(red[:, 3:4], ssacc, axis=mybir.AxisListType.X)
        dma.dma_start(out=out_d, in_=red)

    nc.compile()
    return nc


def _band_host():
    b = np.zeros((257, 128), np.float32)
    for i in range(128):            # BAND_A: i-j in {0,1,2}
        for j in range(128):
            if i - j in (0, 1, 2):
                b[i, j] = 1.0
    for i in range(127):            # BAND_B: i-j in {-1,0,1}
        for j in range(128):
            if i - j in (-1, 0, 1):
                b[128 + i, j] = 1.0
    b[255, 126] = 1.0               # E2 row 0: next-tile row 128c+127
    b[255, 127] = 1.0
    b[256, 127] = 1.0               # E2 row 1 / E1: row 128c+128
    return b


def _get_compiled():
    global _compiled
    if _compiled is None:
        _compiled = _build_nc()
    return _compiled


def _shard_inputs(reconstruction, target):
    band = _band_host()
    if CFG["in_dtype"] == "bf16":
        import ml_dtypes
        dt = ml_dtypes.bfloat16
    else:
        dt = np.float32
    band = band.astype(dt)
    rec = np.asarray(reconstruction).reshape(N_CORES, PLANES, IMG, IMG).astype(dt)
    tgt = np.asarray(target).reshape(N_CORES, PLANES, IMG, IMG).astype(dt)
    return [{"x": np.ascontiguousarray(rec[i]),
             "y": np.ascontiguousarray(tgt[i]),
             "band": band} for i in range(N_CORES)]


def _combine(results):
    sxx = syy = sxy = sss = 0.0
    for i in range(N_CORES):
        red = results[i]["out"].astype(np.float64)
        sxx += red[:, 0].sum()
        syy += red[:, 1].sum()
        sxy += red[:, 2].sum()
        sss += red[:, 3].sum()
    n = float(N_CORES * PLANES * IMG * IMG)
    mse = (sxx + syy - 2.0 * sxy) / n
    ssim_loss = 1.0 - sss / n
    return np.float32(0.8 * mse + 0.2 * ssim_loss)


def run(reconstruction, target, trace=False):
    from concourse.bass_utils import run_bass_kernel_spmd
    nc = _get_compiled()
    in_maps = _shard_inputs(reconstruction, target)
    res = run_bass_kernel_spmd(nc, in_maps, list(range(N_CORES)), trace=trace)
    return _combine(res.results), res


def kernel(reconstruction, target):
    out, _ = run(reconstruction, target, trace=False)
    return out


# revision 26
# speedup vs baseline: 2.0230x; 1.2615x over previous
"""Trainium2 Bass kernel for EnhancedReconstructionLoss (0.8*MSE + 0.2*SSIM-loss).

Sharding: pure data parallel. Batch 32 -> 8 cores x 4 images (12 planes of
512x512 each). Each core computes partial sums (sum x^2, sum y^2, sum x*y,
sum ssim_map*9); host combines into the scalar loss.

Per-core pipeline per 512x512 plane (inputs pre-cast to bf16 on host; all
reductions accumulate in fp32 on-chip):
  - load x,y as a [128, 5, 512] plane tensor of row-shifted tiles
    (tile t holds rows 128t-1..128t+126) so cross-tile vertical-filter edge
    matmuls only need base-partition-0 operands
  - xx=x^2, yy=y^2 (ScalarE, fp32 accum for MSE), xy=x*y (accum), zz=xx+yy
    computed on the whole plane at once
  - vertical 3-tap box filter via TensorE banded matmul -> PSUM (fp32)
  - PSUM->SBUF bf16 copy (hw allows only one PSUM operand per instruction),
    horizontal 3-tap via two shifted-AP adds (DVE, bf16 2x mode)
  - SSIM pointwise tail once per plane at FD=2048 using tensor_tensor (2x)
    and tensor_scalar (4x) ops with the 1/9 pool normalizations folded into
    constants; the host multiplies the ssim sum by 9 at the end
"""

import sys
import numpy as np

for _p in ("/opt/trn_rl_repo", "/root/.axon_site/_ro/trn_rl_repo"):
    if _p not in sys.path:
        sys.path.insert(0, _p)

N_CORES = 8
IMG = 512
PLANES = 12          # 4 images x 3 channels per core
# Tiles are shifted by -1 row: tile t = rows 128t-1..128t+126 (tile 0 only
# 127 rows, tile 4 only row 511). Cross-tile matmul edges then only ever
# need the FIRST rows of the next tile (base partition 0, a hw requirement).
TILE_ROWS = [(0, 127), (127, 255), (255, 383), (383, 511), (511, 512)]
NT = 5
NCHUNK = 4
C1 = 0.01 ** 2
C2 = 0.03 ** 2
EPS = 1e-8

CFG = {
    "dma_eng": "sync",
}

_compiled = None


def _build_nc():
    from contextlib import ExitStack
    import concourse.bass as bass
    import concourse.tile as tile
    from concourse import bacc, mybir

    f32 = mybir.dt.float32
    bf16 = mybir.dt.bfloat16
    Alu = mybir.AluOpType
    Act = mybir.ActivationFunctionType

    nc = bacc.Bacc("TRN2", target_bir_lowering=False, debug=False,
                   enable_asserts=True, num_devices=N_CORES)
    x_d = nc.dram_tensor("x", [PLANES, IMG, IMG], bf16, kind="ExternalInput").ap()
    y_d = nc.dram_tensor("y", [PLANES, IMG, IMG], bf16, kind="ExternalInput").ap()
    band_d = nc.dram_tensor("band", [257, 128], bf16, kind="ExternalInput").ap()
    out_d = nc.dram_tensor("out", [128, 4], f32, kind="ExternalOutput").ap()

    dma = getattr(nc, CFG["dma_eng"])

    with tile.TileContext(nc) as tc, ExitStack() as ctx:
        consts = ctx.enter_context(tc.tile_pool(name="consts", bufs=1))
        inp = ctx.enter_context(tc.tile_pool(name="inp", bufs=2))
        pre = ctx.enter_context(tc.tile_pool(name="pre", bufs=2))
        psum = ctx.enter_context(tc.tile_pool(name="psum", bufs=2, space="PSUM"))
        taps = ctx.enter_context(tc.tile_pool(name="taps", bufs=3))
        s2p = ctx.enter_context(tc.tile_pool(name="s2p", bufs=2))
        tail = ctx.enter_context(tc.tile_pool(name="tail", bufs=2))
        trec = ctx.enter_context(tc.tile_pool(name="trec", bufs=1))
        tshort = ctx.enter_context(tc.tile_pool(name="tshort", bufs=6))
        accs = ctx.enter_context(tc.tile_pool(name="accs", bufs=1))

        band_a = consts.tile([128, 128], bf16, tag="band_a")  # i-j in {0,1,2}
        dma.dma_start(out=band_a, in_=band_d[0:128, :])
        band_b = consts.tile([127, 128], bf16, tag="band_b")  # i-j in {-1,0,1}
        dma.dma_start(out=band_b, in_=band_d[128:255, :])
        e2 = consts.tile([2, 128], bf16, tag="e2")
        dma.dma_start(out=e2, in_=band_d[255:257, :])
        e1 = consts.tile([1, 128], bf16, tag="e1")
        dma.dma_start(out=e1, in_=band_d[256:257, :])

        xxacc = accs.tile([128, PLANES], f32, tag="xxacc")
        yyacc = accs.tile([128, PLANES], f32, tag="yyacc")
        xyacc = accs.tile([128, PLANES], f32, tag="xyacc")
        ssacc = accs.tile([128, PLANES], f32, tag="ssacc")
        for a in (xxacc, yyacc, xyacc, ssacc):
            nc.vector.memset(a, 0.0)

        def load_plane(dst, src_d, p):
            # tile 0: rows 0..126 at partitions 0..126
            dma.dma_start(out=dst[0:127, 0, :], in_=src_d[p, 0:127, :])
            # tiles 1..3: rows 127..510, partition p = row 128t-1+p
            mid = src_d[p, 127:511, :].rearrange("(t r) c -> r t c", r=128)
            dma.dma_start(out=dst[:, 1:4, :], in_=mid)
            # tile 4: row 511 at partition 0
            dma.dma_start(out=dst[0:1, 4, :], in_=src_d[p, 511:512, :])

        for p in range(PLANES):
            # ---- load plane + pre-pool pointwise on the whole plane ----
            xp = inp.tile([128, NT, IMG], bf16, tag="xp")
            yp = inp.tile([128, NT, IMG], bf16, tag="yp")
            if p < 2:
                # zero this pool slot once before its first loads: the pad
                # regions (t0 partition 127, t4 partitions 1..127) are never
                # DMA'd, and slot values persist across the bufs=2 rotation,
                # so derived tensors inherit exact zeros there
                nc.gpsimd.memset(xp, 0.0)
                nc.gpsimd.memset(yp, 0.0)
            load_plane(xp, x_d, p)
            load_plane(yp, y_d, p)

            xxp = pre.tile([128, NT, IMG], bf16, tag="xx")
            nc.scalar.activation(xxp, xp, Act.Square,
                                 accum_out=xxacc[:, p:p + 1])
            yyp = pre.tile([128, NT, IMG], bf16, tag="yy")
            nc.scalar.activation(yyp, yp, Act.Square,
                                 accum_out=yyacc[:, p:p + 1])
            xyp = pre.tile([128, NT, IMG], bf16, tag="xy")
            nc.vector.scalar_tensor_tensor(
                out=xyp, in0=xp, scalar=1.0, in1=yp,
                op0=Alu.mult, op1=Alu.mult, accum_out=xyacc[:, p:p + 1])
            zzp = pre.tile([128, NT, IMG], bf16, tag="zz")
            nc.gpsimd.tensor_add(zzp, xxp, yyp)

            streams = [xp, yp, zzp, xyp]

            # S2 holds the fully box-filtered sums for the whole plane:
            # [partition, stream, chunk, col]
            S2 = s2p.tile([128, 4, NCHUNK, IMG], bf16, tag="S2")

            # ---- per output chunk: vertical matmul + horizontal taps ----
            for c in range(NCHUNK):
                V = psum.tile([128, 4, IMG], f32, tag="V")
                for s, st in enumerate(streams):
                    main_band = band_b if c == 0 else band_a
                    main_rhs = st[0:127, 0, :] if c == 0 else st[:, c, :]
                    if c < NCHUNK - 1:
                        edge = (e2[0:2, :], st[0:2, c + 1, :])
                    else:
                        edge = (e1[0:1, :], st[0:1, c + 1, :])
                    mms = [(main_band, main_rhs), edge]
                    for i, (lhsT, rhs) in enumerate(mms):
                        nc.tensor.matmul(V[:, s, :], lhsT, rhs,
                                         start=(i == 0), stop=(i == len(mms) - 1))

                Vs = taps.tile([128, 4, IMG], bf16, tag="Vs")
                nc.scalar.activation(Vs, V, Act.Copy)

                A = taps.tile([128, 4, IMG], bf16, tag="A")
                nc.vector.memset(A[:, :, 0:1], 0.0)
                nc.vector.tensor_add(A[:, :, 1:IMG], Vs[:, :, 0:IMG - 1],
                                     Vs[:, :, 1:IMG])
                nc.vector.tensor_add(S2[:, :, c, 0:IMG - 1], A[:, :, 0:IMG - 1],
                                     Vs[:, :, 1:IMG])
                nc.scalar.activation(S2[:, :, c, IMG - 1:IMG],
                                     A[:, :, IMG - 1:IMG], Act.Copy)

            # ---- SSIM pointwise tail, whole plane at once (FD = 2048) ----
            # With S = 9*mu (raw 3x3 box sums):
            #   num1 = 2*P/81 + C1            (P = Sx*Sy)
            #   num2 = 2*Sxy/9 + C2 - (num1 - C1)
            #   den1 = qsum/81 + C1           (qsum = Sx^2 + Sy^2)
            #   den2' = 9*den2 = Szz - (qsum/9 - 9*C2)
            #   ssim = num1*num2 / (den1*den2) = 9 * num / dd,  dd = den1*den2'
            # (the x9 is applied on the host)
            FD = NCHUNK * IMG
            Sx = S2[:, 0, :, :]
            Sy = S2[:, 1, :, :]
            Szz = S2[:, 2, :, :]
            Sxy = S2[:, 3, :, :]
            qx = tshort.tile([128, FD], bf16, tag="ts")
            nc.scalar.activation(qx, Sx, Act.Square)
            qy = tshort.tile([128, FD], bf16, tag="ts")
            nc.scalar.activation(qy, Sy, Act.Square)
            qsum = tshort.tile([128, FD], bf16, tag="ts")
            nc.vector.tensor_add(qsum, qx, qy)
            den1 = tail.tile([128, FD], bf16, tag="den1")
            nc.vector.tensor_scalar(out=den1, in0=qsum, scalar1=1.0 / 81.0,
                                    scalar2=C1, op0=Alu.mult, op1=Alu.add)
            U3 = tshort.tile([128, FD], bf16, tag="ts")
            nc.vector.tensor_scalar(out=U3, in0=qsum, scalar1=1.0 / 9.0,
                                    scalar2=-9.0 * C2, op0=Alu.mult, op1=Alu.add)
            den2 = tail.tile([128, FD], bf16, tag="den2")
            nc.vector.tensor_sub(den2, Szz, U3)
            P = tshort.tile([128, FD], bf16, tag="ts")
            nc.vector.tensor_mul(P, Sx, Sy)
            num1 = tail.tile([128, FD], bf16, tag="num1")
            nc.vector.tensor_scalar(out=num1, in0=P, scalar1=2.0 / 81.0,
                                    scalar2=C1, op0=Alu.mult, op1=Alu.add)
            t1 = tshort.tile([128, FD], bf16, tag="ts")
            nc.vector.tensor_scalar(out=t1, in0=Sxy, scalar1=2.0 / 9.0,
                                    scalar2=C2 + C1, op0=Alu.mult, op1=Alu.add)
            num2 = tshort.tile([128, FD], bf16, tag="ts")
            nc.vector.tensor_sub(num2, t1, num1)
            num = tail.tile([128, FD], bf16, tag="num")
            nc.vector.tensor_mul(num, num1, num2)
            dd = trec.tile([128, FD], f32, tag="dd")
            nc.gpsimd.tensor_mul(dd, den1, den2)
            r9 = trec.tile([128, FD], f32, tag="r9")
            nc.vector.reciprocal_approx_fast(out=r9, in_=dd)
            scr = tail.tile([128, FD], bf16, tag="scr")
            nc.vector.scalar_tensor_tensor(
                out=scr, in0=num, scalar=1.0, in1=r9,
                op0=Alu.mult, op1=Alu.mult, accum_out=ssacc[:, p:p + 1])

        red = accs.tile([128, 4], f32, tag="red")
        nc.vector.reduce_sum(red[:, 0:1], xxacc, axis=mybir.AxisListType.X)
        nc.vector.reduce_sum(red[:, 1:2], yyacc, axis=mybir.AxisListType.X)
        nc.vector.reduce_sum(red[:, 2:3], xyacc, axis=mybir.AxisListType.X)
        nc.vector.reduce_sum(red[:, 3:4], ssacc, axis=mybir.AxisListType.X)
        dma.dma_start(out=out_d, in_=red)

    nc.compile()
    return nc


def _band_host():
    b = np.zeros((257, 128), np.float32)
    for i in range(128):            # BAND_A: i-j in {0,1,2}
        for j in range(128):
            if i - j in (0, 1, 2):
                b[i, j] = 1.0
    for i in range(127):            # BAND_B: i-j in {-1,0,1}
        for j in range(128):
            if i - j in (-1, 0, 1):
                b[128 + i, j] = 1.0
    b[255, 126] = 1.0               # E2 row 0: next-tile row 128c+127
    b[255, 127] = 1.0
    b[256, 127] = 1.0               # E2 row 1 / E1: row 128c+128
    return b


def _get_compiled():
    global _compiled
    if _compiled is None:
        _compiled = _build_nc()
    return _compiled


def _shard_inputs(reconstruction, target):
    import ml_dtypes
    dt = ml_dtypes.bfloat16
    band = _band_host().astype(dt)
    rec = np.asarray(reconstruction).reshape(N_CORES, PLANES, IMG, IMG).astype(dt)
    tgt = np.asarray(target).reshape(N_CORES, PLANES, IMG, IMG).astype(dt)
    return [{"x": np.ascontiguousarray(rec[i]),
             "y": np.ascontiguousarray(tgt[i]),
             "band": band} for i in range(N_CORES)]


def _combine(results):
    sxx = syy = sxy = sss = 0.0
    for i in range(N_CORES):
        red = results[i]["out"].astype(np.float64)
        sxx += red[:, 0].sum()
        syy += red[:, 1].sum()
        sxy += red[:, 2].sum()
        sss += red[:, 3].sum()
    n = float(N_CORES * PLANES * IMG * IMG)
    mse = (sxx + syy - 2.0 * sxy) / n
    ssim_loss = 1.0 - 9.0 * sss / n
    return np.float32(0.8 * mse + 0.2 * ssim_loss)


def run(reconstruction, target, trace=False):
    from concourse.bass_utils import run_bass_kernel_spmd
    nc = _get_compiled()
    in_maps = _shard_inputs(reconstruction, target)
    res = run_bass_kernel_spmd(nc, in_maps, list(range(N_CORES)), trace=trace)
    return _combine(res.results), res


def kernel(reconstruction, target):
    out, _ = run(reconstruction, target, trace=False)
    return out
